# revision 14
# baseline (speedup 1.0000x reference)
"""LoRA linear y = x @ (B@A).T computed low-rank: y = (x @ A.T) @ B.T.

Sharding: data-parallel over tokens (B*S = 16384) across 8 NeuronCores,
2048 tokens/core; lora_A / lora_B replicated (tiny). No collectives.

All device I/O in bf16 (rel err ~3.5e-3, gate 2e-2): halves HBM traffic vs
f32 (64 MB -> 32 MB per core; ~358 GB/s/NC => ~90us floor). Host
pre-transposes x into xT chunk layout so the kernel needs NO on-device
transpose.

Tokens run through a 4-stage quarter pipeline (512 tokens each), with the
PE issue order hand-interleaved: mm1 matmuls of quarter q+1 (dep: loads,
which run ahead on the sync ring) are woven between mm2 matmuls of quarter
q (dep: PSUM copy drain), keeping the in-order PE queue dense.

mm1 is 4x column-tiled on the PE array (tile_position=(0,32j)): the four
128-token slabs of a quarter stream concurrently through disjoint
32-column strips (~4x less PE time; measured overlap ~0.26 span ratio).
Each slab's tT lands at PSUM partitions 32j..32j+16 -- exactly where mm2
wants its K=16 operands, so mm2 runs row-positioned (tile_position=(32j,0))
against a B.T replicated at partition offsets 0/32/64/96; no partition
shuffle is ever needed, and one [128,128] copy drains a whole quarter's tT.
With mm1 cheap, per-stage PE work (~32 mm2 MMs) stays at or below the
per-stage DMA time even when the HAM clock gate holds the PE at 1.2 GHz
(K=16 matmuls do not register as PE activity for the HAM, so mm2-heavy
stretches always risk running cold).

DRAM layouts (per core, bf16):
  xtd [4q*4ld*128p, 8j*512t] : xtd[q,ld,p,j,t] = x[tok0+q*512+t, (ld*8+j)*128+p]
  atp [128, 32*16]           : atp[p, c*16+r] = A[r, c*128+p]
  btr [128, 4096]            : btr[32j+r, d] = B.T[r, d]  (x4 replicated)
  ys  [4q*2gg*128p, 2jj*4096]: ys[q,gg,p,jj,:] = y[tok0+q*512+gg*256+jj*128+p, :]

Per quarter: 4x 1MB loads -> mm1 tT[16,128]x4 (K=128, x32 accum, one full
PSUM bank, 2 rotating) -> tT to SBUF bf16 (one [128,128] copy) -> mm2
y[128,512] (K=16) pairs into [128,2,512] PSUM tiles (3 rotating) -> one
[128,1024] f32->bf16 copy per pair (DVE/ACT alternating) -> 2x 2MB stores
(scalar HWDGE ring).
"""

import os
import numpy as np
import ml_dtypes

import concourse.bass as bass
import concourse.mybir as mybir
from concourse.tile import TileContext
from concourse.bass_utils import run_bass_kernel_spmd

N_CORES = 8
B, S, D_IN, D_OUT, R = 4, 4096, 4096, 4096, 16
TOK = B * S
TPC = TOK // N_CORES   # tokens per core: 2048
NQ = 4                 # quarter-pipeline stages per core
TPQ = TPC // NQ        # tokens per quarter: 512
NC_DIN = D_IN // 128   # 32 din chunks
NLD = 4                # x loads per quarter (8 chunks each, 1 MB)
F32 = mybir.dt.float32
BF16 = mybir.dt.bfloat16
NPBF16 = np.dtype(ml_dtypes.bfloat16)


def _split_drain_waits(nc):
    """This walrus build rejects instructions carrying >1 sem wait; hoist
    extra waits onto preceding single-wait NoOps on the same engine."""
    f = nc.m.functions[0]

    def fix_bb(bb):
        insts = getattr(bb, "instructions", None)
        if insts:
            new = []
            for inst in insts:
                si = inst.sync_info
                if si is not None and si.on_wait is not None and len(si.on_wait) > 1:
                    waits = list(si.on_wait)
                    for w in waits[:-1]:
                        d = mybir.InstNoOp(
                            name=nc.get_next_instruction_name(), ins=[], outs=[]
                        )
                        d.engine = inst.engine
                        d.sync_info = mybir.SyncInfo(on_wait=[w], on_update=[])
                        new.append(d)
                    si.on_wait = [waits[-1]]
                    inst.sync_info = si
                new.append(inst)
            bb.instructions[:] = new
        for sub in getattr(bb, "blocks", []) or []:
            fix_bb(sub)

    for blk in f.blocks:
        fix_bb(blk)


def _build():
    nc = bass.Bass("TRN2", target_bir_lowering=False, debug=False, num_devices=N_CORES)
    xtd = nc.declare_dram_parameter("xtd", [NQ * NLD * 128, 8 * TPQ], BF16, isOutput=False)
    atp = nc.declare_dram_parameter("atp", [128, NC_DIN * R], BF16, isOutput=False)
    btr = nc.declare_dram_parameter("btr", [128, D_OUT], BF16, isOutput=False)
    ys = nc.declare_dram_parameter("ys", [NQ * 2 * 128, 2 * D_OUT], BF16, isOutput=True)

    with TileContext(nc) as tc:
        with (
            tc.tile_pool(name="const", bufs=1) as cpool,
            tc.tile_pool(name="x", bufs=int(os.environ.get("XB", "6"))) as xpool,
            tc.tile_pool(name="t", bufs=2) as tpool,
            tc.tile_pool(name="y", bufs=int(os.environ.get("YB", "3"))) as ypool,
            tc.tile_pool(name="t_ps", bufs=2, space="PSUM") as tpsum,
            tc.tile_pool(name="y_ps", bufs=int(os.environ.get("YPB", "3")), space="PSUM") as ypsum,
        ):
            at_sb = cpool.tile([128, NC_DIN * R], BF16)
            nc.scalar.dma_start(out=at_sb[:], in_=atp[:])
            bt_sb = cpool.tile([128, D_OUT], BF16)
            nc.scalar.dma_start(out=bt_sb[:], in_=btr[:])

            xts = {}

            def issue_loads(q):
                xts[q] = []
                for ld in range(NLD):
                    xt = xpool.tile([128, 8, TPQ], BF16)
                    nc.sync.dma_start(
                        out=xt[:],
                        in_=xtd[(q * NLD + ld) * 128 : (q * NLD + ld + 1) * 128, :],
                    )
                    xts[q].append(xt)

            def mm1_chunk(q, tps, ld, j):
                # one din chunk c for all 4 col-tiled 128-token slabs
                c = ld * 8 + j
                for ct in range(4):
                    nc.tensor.matmul(
                        tps[32 * ct : 32 * ct + R, 0:128],
                        at_sb[:, c * R : (c + 1) * R],
                        xts[q][ld][:, j, ct * 128 : (ct + 1) * 128],
                        start=(c == 0),
                        stop=(c == NC_DIN - 1),
                        tile_position=(0, 32 * ct),
                    )

            # eighth-pipeline (256 tokens each): e -> quarter q=e//2,
            # token-half h=e%2. Loads stay quarterly; mm1 is 2x col-tiled
            # per eighth (slabs of 128 tokens at partition bases 0/32).
            def mm1_chunk_e(e, tps, ld, j):
                c = ld * 8 + j
                q, h = e // 2, e % 2
                for ct in range(2):
                    nc.tensor.matmul(
                        tps[32 * ct : 32 * ct + R, 0:128],
                        at_sb[:, c * R : (c + 1) * R],
                        xts[q][ld][:, j, h * 256 + ct * 128 : h * 256 + (ct + 1) * 128],
                        start=(c == 0),
                        stop=(c == NC_DIN - 1),
                        tile_position=(0, 32 * ct),
                    )

            NE = 2 * NQ
            issue_loads(0)
            issue_loads(1)
            tps_e = {0: tpsum.tile([128, 512], F32, name="tps")}
            for ld in range(NLD):
                for j in range(8):
                    mm1_chunk_e(0, tps_e[0], ld, j)

            for e in range(NE):
                q = e // 2
                if e % 2 == 0 and q + 2 < NQ:
                    issue_loads(q + 2)
                # tT(e) PSUM -> SBUF bf16 (one copy; frees the bank for e+2)
                t_sb = tpool.tile([64, 128], BF16)
                if e % 2 == 0:
                    nc.vector.tensor_copy(out=t_sb[:], in_=tps_e[e][0:64, 0:128])
                else:
                    nc.scalar.activation(
                        out=t_sb[:], in_=tps_e[e][0:64, 0:128],
                        func=mybir.ActivationFunctionType.Identity,
                    )
                if e + 1 < NE:
                    tps_e[e + 1] = tpsum.tile([128, 512], F32, name="tps")

                # mm2(e): 8 units of (2 MM + 1 fat copy), woven with
                # mm1(e+1): 4 chunk-groups (8 col-tiled MMs) per unit
                mm1_iter = iter(
                    [(ld, j) for ld in range(NLD) for j in range(8)]
                    if e + 1 < NE
                    else []
                )
                u = 0
                y_sb = ypool.tile([128, 2, D_OUT], BF16)
                for jj in range(2):
                    ct = jj  # which 128-token slab / col-tile
                    for nbp in range(4):
                        yp = ypsum.tile([128, 2, 512], F32)
                        for k in range(2):
                            nc.tensor.matmul(
                                yp[:, k, :],
                                t_sb[32 * ct : 32 * ct + R, :],
                                bt_sb[32 * ct : 32 * ct + R, (nbp * 2 + k) * 512 : (nbp * 2 + k + 1) * 512],
                                start=True,
                                stop=True,
                                tile_position=(32 * ct, 0),
                            )
                        for _ in range(4):
                            nxt = next(mm1_iter, None)
                            if nxt is not None:
                                mm1_chunk_e(e + 1, tps_e[e + 1], *nxt)
                        if u % 2 == 0:
                            nc.vector.tensor_copy(
                                out=y_sb[:, jj, nbp * 1024 : (nbp + 1) * 1024],
                                in_=yp[:],
                            )
                        else:
                            nc.scalar.activation(
                                out=y_sb[:, jj, nbp * 1024 : (nbp + 1) * 1024],
                                in_=yp[:],
                                func=mybir.ActivationFunctionType.Identity,
                            )
                        u += 1
                nc.scalar.dma_start(
                    out=ys[e * 128 : (e + 1) * 128, :], in_=y_sb[:]
                )

    _split_drain_waits(nc)
    return nc


_NC = None


def _get_nc():
    global _NC
    if _NC is None:
        _NC = _build()
    return _NC


def _prep_inputs(x, lora_A, lora_B):
    x_flat = np.asarray(x, dtype=np.float32).reshape(TOK, D_IN)
    xb16 = x_flat.astype(NPBF16).view(np.uint16)
    A = np.asarray(lora_A, dtype=np.float32)
    Bm = np.asarray(lora_B, dtype=np.float32)
    xtds = []
    for i in range(N_CORES):
        # [q*t, ld, j, p] -> [q, ld, p, j, t]
        xc = xb16[i * TPC : (i + 1) * TPC].reshape(NQ, TPQ, NLD, 8, 128)
        xtd = (
            np.ascontiguousarray(xc.transpose(0, 2, 4, 3, 1))
            .reshape(NQ * NLD * 128, 8 * TPQ)
            .view(NPBF16)
        )
        xtds.append(xtd)
    # atp[p, c*R + r] = A[r, c*128 + p]
    atp = np.ascontiguousarray(
        A.T.reshape(NC_DIN, 128, R).transpose(1, 0, 2).reshape(128, NC_DIN * R)
    ).astype(NPBF16)
    # btr[32j + r, :] = B.T[r, :], replicated at partition offsets 0/32/64/96
    btv = np.ascontiguousarray(Bm.T).astype(NPBF16)
    btrm = np.zeros((128, D_OUT), dtype=NPBF16)
    for j in range(4):
        btrm[32 * j : 32 * j + R] = btv
    return xtds, atp, btrm


def kernel(x, lora_A, lora_B, _trace=False, _trace_kwargs=None):
    nc = _get_nc()
    xtds, atp, btrm = _prep_inputs(x, lora_A, lora_B)
    in_maps = [{"xtd": xtds[i], "atp": atp, "btr": btrm} for i in range(N_CORES)]
    res = run_bass_kernel_spmd(
        nc, in_maps, list(range(N_CORES)), trace=_trace, **(_trace_kwargs or {})
    )
    out = np.empty((TOK, D_OUT), dtype=np.float32)
    for i in range(N_CORES):
        # ys [q, gg, p, jj, d] -> tokens q*512 + gg*256 + jj*128 + p
        u = (
            np.asarray(res.results[i]["ys"])
            .view(np.uint16)
            .reshape(NQ, 2, 128, 2, D_OUT)
            .transpose(0, 1, 3, 2, 4)
        )
        out[i * TPC : (i + 1) * TPC] = (
            np.ascontiguousarray(u).reshape(TPC, D_OUT).view(NPBF16).astype(np.float32)
        )
    out = out.reshape(B, S, D_OUT)
    if _trace:
        return out, res
    return out


# revision 15
# speedup vs baseline: 1.0821x; 1.0821x over previous
"""LoRA linear y = x @ (B@A).T computed low-rank: y = (x @ A.T) @ B.T.

Sharding: data-parallel over tokens (B*S = 16384) across 8 NeuronCores,
2048 tokens/core; lora_A / lora_B replicated (tiny). No collectives.

All device I/O in bf16 (rel err ~3.5e-3, gate 2e-2): halves HBM traffic vs
f32 (64 MB -> 32 MB per core; ~358 GB/s/NC => ~90us floor). Host
pre-transposes x into xT chunk layout so the kernel needs NO on-device
transpose.

Tokens run through a 4-stage quarter pipeline (512 tokens each), with the
PE issue order hand-interleaved: mm1 matmuls of quarter q+1 (dep: loads,
which run ahead on the sync ring) are woven between mm2 matmuls of quarter
q (dep: PSUM copy drain), keeping the in-order PE queue dense.

mm1 is 4x column-tiled on the PE array (tile_position=(0,32j)): the four
128-token slabs of a quarter stream concurrently through disjoint
32-column strips (~4x less PE time; measured overlap ~0.26 span ratio).
Each slab's tT lands at PSUM partitions 32j..32j+16 -- exactly where mm2
wants its K=16 operands, so mm2 runs row-positioned (tile_position=(32j,0))
against a B.T replicated at partition offsets 0/32/64/96; no partition
shuffle is ever needed, and one [128,128] copy drains a whole quarter's tT.
With mm1 cheap, per-stage PE work (~32 mm2 MMs) stays at or below the
per-stage DMA time even when the HAM clock gate holds the PE at 1.2 GHz
(K=16 matmuls do not register as PE activity for the HAM, so mm2-heavy
stretches always risk running cold).

DRAM layouts (per core, bf16):
  xtd [4q*4ld*128p, 8j*512t] : xtd[q,ld,p,j,t] = x[tok0+q*512+t, (ld*8+j)*128+p]
  atp [128, 32*16]           : atp[p, c*16+r] = A[r, c*128+p]
  btr [128, 4096]            : btr[32j+r, d] = B.T[r, d]  (x4 replicated)
  ys  [4q*2gg*128p, 2jj*4096]: ys[q,gg,p,jj,:] = y[tok0+q*512+gg*256+jj*128+p, :]

Per quarter: 4x 1MB loads -> mm1 tT[16,128]x4 (K=128, x32 accum, one full
PSUM bank, 2 rotating) -> tT to SBUF bf16 (one [128,128] copy) -> mm2
y[128,512] (K=16) pairs into [128,2,512] PSUM tiles (3 rotating) -> one
[128,1024] f32->bf16 copy per pair (DVE/ACT alternating) -> 2x 2MB stores
(scalar HWDGE ring).
"""

import os
import numpy as np
import ml_dtypes

import concourse.bass as bass
import concourse.mybir as mybir
from concourse.tile import TileContext
from concourse.bass_utils import run_bass_kernel_spmd

N_CORES = 8
B, S, D_IN, D_OUT, R = 4, 4096, 4096, 4096, 16
TOK = B * S
TPC = TOK // N_CORES   # tokens per core: 2048
NQ = 4                 # quarter-pipeline stages per core
TPQ = TPC // NQ        # tokens per quarter: 512
NC_DIN = D_IN // 128   # 32 din chunks
NLD = 4                # x loads per quarter (8 chunks each, 1 MB)
F32 = mybir.dt.float32
BF16 = mybir.dt.bfloat16
NPBF16 = np.dtype(ml_dtypes.bfloat16)


def _split_drain_waits(nc):
    """This walrus build rejects instructions carrying >1 sem wait; hoist
    extra waits onto preceding single-wait NoOps on the same engine."""
    f = nc.m.functions[0]

    def fix_bb(bb):
        insts = getattr(bb, "instructions", None)
        if insts:
            new = []
            for inst in insts:
                si = inst.sync_info
                if si is not None and si.on_wait is not None and len(si.on_wait) > 1:
                    waits = list(si.on_wait)
                    for w in waits[:-1]:
                        d = mybir.InstNoOp(
                            name=nc.get_next_instruction_name(), ins=[], outs=[]
                        )
                        d.engine = inst.engine
                        d.sync_info = mybir.SyncInfo(on_wait=[w], on_update=[])
                        new.append(d)
                    si.on_wait = [waits[-1]]
                    inst.sync_info = si
                new.append(inst)
            bb.instructions[:] = new
        for sub in getattr(bb, "blocks", []) or []:
            fix_bb(sub)

    for blk in f.blocks:
        fix_bb(blk)


def _build():
    nc = bass.Bass("TRN2", target_bir_lowering=False, debug=False, num_devices=N_CORES)
    xtd = nc.declare_dram_parameter("xtd", [NQ * NLD * 128, 8 * TPQ], BF16, isOutput=False)
    atp = nc.declare_dram_parameter("atp", [128, NC_DIN * R], BF16, isOutput=False)
    btr = nc.declare_dram_parameter("btr", [128, D_OUT], BF16, isOutput=False)
    ys = nc.declare_dram_parameter("ys", [NQ * 2 * 128, 2 * D_OUT], BF16, isOutput=True)

    with TileContext(nc) as tc:
        with (
            tc.tile_pool(name="const", bufs=1) as cpool,
            tc.tile_pool(name="x", bufs=int(os.environ.get("XB", "8"))) as xpool,
            tc.tile_pool(name="t", bufs=2) as tpool,
            tc.tile_pool(name="y", bufs=int(os.environ.get("YB", "4"))) as ypool,
            tc.tile_pool(name="t_ps", bufs=2, space="PSUM") as tpsum,
            tc.tile_pool(name="y_ps", bufs=int(os.environ.get("YPB", "3")), space="PSUM") as ypsum,
        ):
            at_sb = cpool.tile([128, NC_DIN * R], BF16)
            nc.scalar.dma_start(out=at_sb[:], in_=atp[:])
            bt_sb = cpool.tile([128, D_OUT], BF16)
            nc.scalar.dma_start(out=bt_sb[:], in_=btr[:])

            xts = {}

            def issue_loads(q):
                xts[q] = []
                for ld in range(NLD):
                    xt = xpool.tile([128, 8, TPQ], BF16)
                    nc.sync.dma_start(
                        out=xt[:],
                        in_=xtd[(q * NLD + ld) * 128 : (q * NLD + ld + 1) * 128, :],
                    )
                    xts[q].append(xt)

            def mm1_chunk(q, tps, ld, j):
                # one din chunk c for all 4 col-tiled 128-token slabs
                c = ld * 8 + j
                for ct in range(4):
                    nc.tensor.matmul(
                        tps[32 * ct : 32 * ct + R, 0:128],
                        at_sb[:, c * R : (c + 1) * R],
                        xts[q][ld][:, j, ct * 128 : (ct + 1) * 128],
                        start=(c == 0),
                        stop=(c == NC_DIN - 1),
                        tile_position=(0, 32 * ct),
                    )

            # prologue: loads q0/q1, mm1(q0)
            issue_loads(0)
            issue_loads(1)
            tps_q = {0: tpsum.tile([128, 512], F32, name="tps")}
            for ld in range(NLD):
                for j in range(8):
                    mm1_chunk(0, tps_q[0], ld, j)

            for q in range(NQ):
                if q + 2 < NQ:
                    issue_loads(q + 2)
                # tT(q) PSUM -> SBUF bf16 (one copy; frees the bank for q+2)
                t_sb = tpool.tile([128, 128], BF16)
                if q % 2 == 0:
                    nc.vector.tensor_copy(out=t_sb[:], in_=tps_q[q][:, 0:128])
                else:
                    nc.scalar.activation(
                        out=t_sb[:], in_=tps_q[q][:, 0:128],
                        func=mybir.ActivationFunctionType.Identity,
                    )
                if q + 1 < NQ:
                    tps_q[q + 1] = tpsum.tile([128, 512], F32, name="tps")

                # mm2(q): 16 units of (2 MM + 1 fat copy), woven with
                # mm1(q+1): 2 chunk-groups (8 col-tiled MMs) per unit
                mm1_iter = iter(
                    [(ld, j) for ld in range(NLD) for j in range(8)]
                    if q + 1 < NQ
                    else []
                )
                u = 0
                for gg in range(2):
                    y_sb = ypool.tile([128, 2, D_OUT], BF16)
                    for jj in range(2):
                        ct = gg * 2 + jj  # which 128-token slab / col-tile
                        for nbp in range(4):
                            yp = ypsum.tile([128, 2, 512], F32)
                            for k in range(2):
                                nc.tensor.matmul(
                                    yp[:, k, :],
                                    t_sb[32 * ct : 32 * ct + R, :],
                                    bt_sb[32 * ct : 32 * ct + R, (nbp * 2 + k) * 512 : (nbp * 2 + k + 1) * 512],
                                    start=True,
                                    stop=True,
                                    tile_position=(32 * ct, 0),
                                )
                            for _ in range(2):
                                nxt = next(mm1_iter, None)
                                if nxt is not None:
                                    mm1_chunk(q + 1, tps_q[q + 1], *nxt)
                            if u % 2 == 0:
                                nc.vector.tensor_copy(
                                    out=y_sb[:, jj, nbp * 1024 : (nbp + 1) * 1024],
                                    in_=yp[:],
                                )
                            else:
                                nc.scalar.activation(
                                    out=y_sb[:, jj, nbp * 1024 : (nbp + 1) * 1024],
                                    in_=yp[:],
                                    func=mybir.ActivationFunctionType.Identity,
                                )
                            u += 1
                    nc.scalar.dma_start(
                        out=ys[(q * 2 + gg) * 128 : (q * 2 + gg + 1) * 128, :],
                        in_=y_sb[:],
                    )

    _split_drain_waits(nc)
    return nc


_NC = None


def _get_nc():
    global _NC
    if _NC is None:
        _NC = _build()
    return _NC


def _prep_inputs(x, lora_A, lora_B):
    x_flat = np.asarray(x, dtype=np.float32).reshape(TOK, D_IN)
    xb16 = x_flat.astype(NPBF16).view(np.uint16)
    A = np.asarray(lora_A, dtype=np.float32)
    Bm = np.asarray(lora_B, dtype=np.float32)
    xtds = []
    for i in range(N_CORES):
        # [q*t, ld, j, p] -> [q, ld, p, j, t]
        xc = xb16[i * TPC : (i + 1) * TPC].reshape(NQ, TPQ, NLD, 8, 128)
        xtd = (
            np.ascontiguousarray(xc.transpose(0, 2, 4, 3, 1))
            .reshape(NQ * NLD * 128, 8 * TPQ)
            .view(NPBF16)
        )
        xtds.append(xtd)
    # atp[p, c*R + r] = A[r, c*128 + p]
    atp = np.ascontiguousarray(
        A.T.reshape(NC_DIN, 128, R).transpose(1, 0, 2).reshape(128, NC_DIN * R)
    ).astype(NPBF16)
    # btr[32j + r, :] = B.T[r, :], replicated at partition offsets 0/32/64/96
    btv = np.ascontiguousarray(Bm.T).astype(NPBF16)
    btrm = np.zeros((128, D_OUT), dtype=NPBF16)
    for j in range(4):
        btrm[32 * j : 32 * j + R] = btv
    return xtds, atp, btrm


def kernel(x, lora_A, lora_B, _trace=False, _trace_kwargs=None):
    nc = _get_nc()
    xtds, atp, btrm = _prep_inputs(x, lora_A, lora_B)
    in_maps = [{"xtd": xtds[i], "atp": atp, "btr": btrm} for i in range(N_CORES)]
    res = run_bass_kernel_spmd(
        nc, in_maps, list(range(N_CORES)), trace=_trace, **(_trace_kwargs or {})
    )
    out = np.empty((TOK, D_OUT), dtype=np.float32)
    for i in range(N_CORES):
        # ys [q, gg, p, jj, d] -> tokens q*512 + gg*256 + jj*128 + p
        u = (
            np.asarray(res.results[i]["ys"])
            .view(np.uint16)
            .reshape(NQ, 2, 128, 2, D_OUT)
            .transpose(0, 1, 3, 2, 4)
        )
        out[i * TPC : (i + 1) * TPC] = (
            np.ascontiguousarray(u).reshape(TPC, D_OUT).view(NPBF16).astype(np.float32)
        )
    out = out.reshape(B, S, D_OUT)
    if _trace:
        return out, res
    return out
